# revision 6
# baseline (speedup 1.0000x reference)
"""Trainium2 Bass kernel for nn_DETR_8478265442414 (span-based coref /
DETR-style slot-attention head).

Math restructure vs the reference:
  - The masked mention-word softmax + head projection is computed via prefix
    sums in projected space: head_term[s] = (C[e] - Cx[s]) / (c[e] - cx[s])
    with C = inclusive prefix of exp(wa) * (doc @ W4) over positions, Cx the
    exclusive prefix.  Softmax is shift-invariant, so the max-subtraction is
    dropped (|wa| stays small for this problem's scale).
  - start/end embeddings are projected before gathering: P = doc @ [W1|W2],
    then per-span row gathers, so the [30285, 3092] GEMM never materializes.
Sharding: spans (S=30285 -> 8 x 3840, zero-padded) across 8 cores; the 100
slots are replicated; one [100, 257] AllReduce per slot-attention iteration
combines unnormalized slot updates + attention row-sums over the span axis.
"""

import numpy as np

import concourse.bass as bass
import concourse.tile as tile
from concourse import mybir
from concourse.bass_utils import run_bass_kernel_spmd
from concourse.masks import make_identity

F32 = mybir.dt.float32
I32 = mybir.dt.int32
AF = mybir.ActivationFunctionType
OP = mybir.AluOpType

N = 1024
H = 1024
D = 256
Q = 100
L = 30
WD = 20
S = 30285
NCORES = 8
SC = 3840          # spans per core (padded)
NT = SC // 128     # 30 span tiles of 128 per core
LN_EPS = 1e-5
ATTN_EPS = 1e-8
SCALE = float(D) ** -0.5


def _split_excess_waits(nc):
    """walrus CoreV3 in this toolchain encodes at most one sync wait per
    instruction (two for EventSemaphore).  Hoist excess waits onto
    same-engine NoOps placed immediately before the instruction; per-engine
    program order makes that equivalent."""
    n = 0
    for f in nc.m.functions:
        for blk in f.blocks:
            out = []
            changed = False
            for inst in blk.instructions:
                si = inst.sync_info
                waits = list(si.on_wait) if si is not None and si.on_wait else []
                cap = 2 if "EventSemaphore" in str(inst.opcode) else 1
                if len(waits) > cap:
                    for w in waits[:-cap]:
                        nop = mybir.InstNoOp(name=f"waitsplit-{n}", ins=[], outs=[])
                        n += 1
                        nop.engine = inst.engine
                        nop.sync_info = mybir.SyncInfo(on_wait=[w], on_update=[])
                        out.append(nop)
                    si.on_wait = waits[-cap:]
                    changed = True
                out.append(inst)
            if changed:
                blk.instructions = out


def _bcast_ap(handle, parts, width):
    """[width] 1-D DRAM tensor -> [parts, width] stride-0 partition AP."""
    ap = handle.ap()
    return bass.AP(tensor=ap.tensor, offset=0, ap=[[0, parts], [1, width]])


def _col_ap(handle, offset, rows):
    """rows consecutive elements of a 1-D DRAM tensor as a [rows, 1] AP."""
    ap = handle.ap()
    return bass.AP(tensor=ap.tensor, offset=offset, ap=[[1, rows], [1, 1]])


def build_nc():
    nc = bass.Bass(num_devices=NCORES)

    doc = nc.dram_tensor("doc", [N, H], F32, kind="ExternalInput")
    w_cat = nc.dram_tensor("w_cat", [H, 3 * D], F32, kind="ExternalInput")
    wwa = nc.dram_tensor("wwa", [H, 1], F32, kind="ExternalInput")
    bwa = nc.dram_tensor("bwa", [1], F32, kind="ExternalInput")
    wemb = nc.dram_tensor("wemb", [L, WD], F32, kind="ExternalInput")
    w3 = nc.dram_tensor("w3", [WD, D], F32, kind="ExternalInput")
    b_span = nc.dram_tensor("b_span", [D], F32, kind="ExternalInput")
    slots_q = nc.dram_tensor("slots_q", [Q, D], F32, kind="ExternalInput")
    wq = nc.dram_tensor("wq", [D, D], F32, kind="ExternalInput")
    bq = nc.dram_tensor("bq", [D], F32, kind="ExternalInput")
    wk = nc.dram_tensor("wk", [D, D], F32, kind="ExternalInput")
    bk = nc.dram_tensor("bk", [D], F32, kind="ExternalInput")
    wv = nc.dram_tensor("wv", [D, D], F32, kind="ExternalInput")
    bv = nc.dram_tensor("bv", [D], F32, kind="ExternalInput")
    w_ih = nc.dram_tensor("w_ih", [3 * D, D], F32, kind="ExternalInput")
    w_hh = nc.dram_tensor("w_hh", [3 * D, D], F32, kind="ExternalInput")
    b_ih = nc.dram_tensor("b_ih", [3 * D], F32, kind="ExternalInput")
    b_hh = nc.dram_tensor("b_hh", [3 * D], F32, kind="ExternalInput")
    w_mlp1 = nc.dram_tensor("w_mlp1", [D, 2 * D], F32, kind="ExternalInput")
    b_mlp1 = nc.dram_tensor("b_mlp1", [2 * D], F32, kind="ExternalInput")
    w_mlp2 = nc.dram_tensor("w_mlp2", [2 * D, D], F32, kind="ExternalInput")
    b_mlp2 = nc.dram_tensor("b_mlp2", [D], F32, kind="ExternalInput")
    g_in = nc.dram_tensor("g_in", [D], F32, kind="ExternalInput")
    beta_in = nc.dram_tensor("beta_in", [D], F32, kind="ExternalInput")
    g_sl = nc.dram_tensor("g_sl", [D], F32, kind="ExternalInput")
    beta_sl = nc.dram_tensor("beta_sl", [D], F32, kind="ExternalInput")
    g_ff = nc.dram_tensor("g_ff", [D], F32, kind="ExternalInput")
    beta_ff = nc.dram_tensor("beta_ff", [D], F32, kind="ExternalInput")
    idx_start = nc.dram_tensor("idx_start", [128, NT], I32, kind="ExternalInput")
    idx_end = nc.dram_tensor("idx_end", [128, NT], I32, kind="ExternalInput")
    idx_w = nc.dram_tensor("idx_w", [128, NT], I32, kind="ExternalInput")

    inputs_out = nc.dram_tensor("inputs_out", [SC, D], F32, kind="ExternalOutput")
    logits_out = nc.dram_tensor("logits_out", [Q, SC], F32, kind="ExternalOutput")

    # gather tables (raw internal DRAM: indirect DMA needs offset-0 APs)
    t_start = nc.dram_tensor("t_start", [N, 2 * D + 1], F32)
    t_end = nc.dram_tensor("t_end", [N, 2 * D + 1], F32)
    t_w = nc.dram_tensor("t_w", [L, D], F32)

    with tile.TileContext(nc) as tc:
        with (
            tc.tile_pool(name="const", bufs=1) as const,
            tc.tile_pool(name="persist", bufs=1) as persist,
        ):
            # ---------------- constants / weights in SBUF ----------------
            ident = const.tile([128, 128], F32)
            make_identity(nc, ident[:])
            lt = const.tile([128, 128], F32)  # lt[x, y] = 1 iff x <= y
            nc.gpsimd.memset(lt[:], 0.0)
            nc.gpsimd.affine_select(
                out=lt[:], in_=lt[:], compare_op=OP.is_gt, fill=1.0,
                base=0, pattern=[[-1, 128]], channel_multiplier=1,
            )
            ones_t = const.tile([128, 128], F32)
            nc.vector.memset(ones_t[:], 1.0)
            eps_sb = const.tile([128, 1], F32)
            nc.vector.memset(eps_sb[:], LN_EPS)

            wwa_sb = [const.tile([128, 1], F32, tag=f"wwa{k}", name=f"wwa{k}") for k in range(8)]
            for k in range(8):
                nc.sync.dma_start(out=wwa_sb[k][:], in_=wwa[k * 128:(k + 1) * 128, :])
            bwa_sb = const.tile([128, 1], F32)
            nc.sync.dma_start(out=bwa_sb[:], in_=_bcast_ap(bwa, 128, 1))

            wemb_sb = const.tile([L, WD], F32)
            nc.sync.dma_start(out=wemb_sb[:], in_=wemb[:, :])
            w3_sb = const.tile([WD, D], F32)
            nc.sync.dma_start(out=w3_sb[:], in_=w3[:, :])
            bspan_bc = const.tile([128, D], F32)
            nc.sync.dma_start(out=bspan_bc[:], in_=_bcast_ap(b_span, 128, D))

            wk_sb = [const.tile([128, D], F32, tag=f"wk{k}", name=f"wk{k}") for k in range(2)]
            wv_sb = [const.tile([128, D], F32, tag=f"wv{k}", name=f"wv{k}") for k in range(2)]
            wq_sb = [const.tile([128, D], F32, tag=f"wq{k}", name=f"wq{k}") for k in range(2)]
            for k in range(2):
                nc.sync.dma_start(out=wk_sb[k][:], in_=wk[k * 128:(k + 1) * 128, :])
                nc.sync.dma_start(out=wv_sb[k][:], in_=wv[k * 128:(k + 1) * 128, :])
                nc.sync.dma_start(out=wq_sb[k][:], in_=wq[k * 128:(k + 1) * 128, :])
            bkT = [const.tile([128, 1], F32, tag=f"bkT{m}", name=f"bkT{m}") for m in range(2)]
            bqT = [const.tile([128, 1], F32, tag=f"bqT{m}", name=f"bqT{m}") for m in range(2)]
            for m in range(2):
                nc.sync.dma_start(out=bkT[m][:], in_=_col_ap(bk, m * 128, 128))
                nc.sync.dma_start(out=bqT[m][:], in_=_col_ap(bq, m * 128, 128))
            bv_bc = const.tile([128, D], F32)
            nc.sync.dma_start(out=bv_bc[:], in_=_bcast_ap(bv, 128, D))

            g_in_bc = const.tile([128, D], F32)
            beta_in_bc = const.tile([128, D], F32)
            g_sl_bc = const.tile([128, D], F32)
            beta_sl_bc = const.tile([128, D], F32)
            g_ff_bc = const.tile([128, D], F32)
            beta_ff_bc = const.tile([128, D], F32)
            for t, h in (
                (g_in_bc, g_in), (beta_in_bc, beta_in), (g_sl_bc, g_sl),
                (beta_sl_bc, beta_sl), (g_ff_bc, g_ff), (beta_ff_bc, beta_ff),
            ):
                nc.sync.dma_start(out=t[:], in_=_bcast_ap(h, 128, D))

            bih_bc = const.tile([128, 3 * D], F32)
            nc.sync.dma_start(out=bih_bc[:], in_=_bcast_ap(b_ih, 128, 3 * D))
            bhh_bc = const.tile([128, 3 * D], F32)
            nc.sync.dma_start(out=bhh_bc[:], in_=_bcast_ap(b_hh, 128, 3 * D))
            bm1_bc = const.tile([128, 2 * D], F32)
            nc.sync.dma_start(out=bm1_bc[:], in_=_bcast_ap(b_mlp1, 128, 2 * D))
            bm2_bc = const.tile([128, D], F32)
            nc.sync.dma_start(out=bm2_bc[:], in_=_bcast_ap(b_mlp2, 128, D))

            wm1_sb = [const.tile([128, 2 * D], F32, tag=f"wm1{k}", name=f"wm1{k}") for k in range(2)]
            for k in range(2):
                nc.sync.dma_start(out=wm1_sb[k][:], in_=w_mlp1[k * 128:(k + 1) * 128, :])
            wm2_sb = [const.tile([128, D], F32, tag=f"wm2{k}", name=f"wm2{k}") for k in range(4)]
            for k in range(4):
                nc.sync.dma_start(out=wm2_sb[k][:], in_=w_mlp2[k * 128:(k + 1) * 128, :])

            idxs_sb = const.tile([128, NT], I32)
            idxe_sb = const.tile([128, NT], I32)
            idxw_sb = const.tile([128, NT], I32)
            nc.sync.dma_start(out=idxs_sb[:], in_=idx_start[:, :])
            nc.sync.dma_start(out=idxe_sb[:], in_=idx_end[:, :])
            nc.sync.dma_start(out=idxw_sb[:], in_=idx_w[:, :])

            # transposed GRU weights: w_ihT/w_hhT as 2 tiles of [128, 768]
            wihT = [persist.tile([128, 3 * D], F32, tag=f"wihT{c}", name=f"wihT{c}") for c in range(2)]
            whhT = [persist.tile([128, 3 * D], F32, tag=f"whhT{c}", name=f"whhT{c}") for c in range(2)]
            with (
                tc.tile_pool(name="ph0", bufs=2) as ph0,
                tc.tile_pool(name="ph0ps", bufs=2, space="PSUM") as ph0ps,
            ):
                for src, dstT in ((w_ih, wihT), (w_hh, whhT)):
                    for r in range(6):
                        rt = ph0.tile([128, D], F32, tag="gru_ld")
                        nc.sync.dma_start(out=rt[:], in_=src[r * 128:(r + 1) * 128, :])
                        for c in range(2):
                            ps = ph0ps.tile([128, 128], F32, tag="gru_tr")
                            nc.tensor.transpose(
                                out=ps[:], in_=rt[:, c * 128:(c + 1) * 128],
                                identity=ident[:],
                            )
                            nc.vector.tensor_copy(
                                out=dstT[c][:, r * 128:(r + 1) * 128], in_=ps[:]
                            )

            # ---------------- phase 1: doc projections + prefix ----------
            with (
                tc.tile_pool(name="ph1", bufs=2) as ph1,
                tc.tile_pool(name="ph1doc", bufs=1) as ph1doc,
                tc.tile_pool(name="ph1tr", bufs=2, space="PSUM") as ph1tr,
                tc.tile_pool(name="ph1wa", bufs=1, space="PSUM") as ph1wa,
                tc.tile_pool(name="ph1pa", bufs=2, space="PSUM") as ph1pa,
                tc.tile_pool(name="ph1pb", bufs=1, space="PSUM") as ph1pb,
                tc.tile_pool(name="ph1c", bufs=1, space="PSUM") as ph1c,
            ):
                wcat_sb = [ph1doc.tile([128, 3 * D], F32, tag=f"wcat{k}", name=f"wcat{k}") for k in range(8)]
                for k in range(8):
                    nc.sync.dma_start(out=wcat_sb[k][:],
                                      in_=w_cat[k * 128:(k + 1) * 128, :])
                docT = [ph1doc.tile([128, N], F32, tag=f"docT{i}", name=f"docT{i}") for i in range(8)]
                for i in range(8):
                    for j in range(8):
                        dblk = ph1.tile([128, 128], F32, tag="dblk")
                        nc.sync.dma_start(
                            out=dblk[:],
                            in_=doc[j * 128:(j + 1) * 128, i * 128:(i + 1) * 128],
                        )
                        ps = ph1tr.tile([128, 128], F32, tag="tr")
                        nc.tensor.transpose(out=ps[:], in_=dblk[:], identity=ident[:])
                        nc.vector.tensor_copy(
                            out=docT[i][:, j * 128:(j + 1) * 128], in_=ps[:]
                        )

                # word attention logits -> E = exp(wa + b)
                e_sb = ph1doc.tile([128, 8], F32, tag="e_sb")
                for j in range(8):
                    ps = ph1wa.tile([128, 1], F32, tag="wa_ps")
                    for k in range(8):
                        nc.tensor.matmul(
                            out=ps[:], lhsT=docT[k][:, j * 128:(j + 1) * 128],
                            rhs=wwa_sb[k][:], start=(k == 0), stop=(k == 7),
                        )
                    nc.scalar.activation(
                        out=e_sb[:, j:j + 1], in_=ps[:], func=AF.Exp,
                        bias=bwa_sb[:, 0:1], scale=1.0,
                    )

                # P = doc @ [W1|W2|W4]; EP4ext = [E*P4, E]
                ep4 = [ph1doc.tile([128, D + 1], F32, tag=f"ep4_{j}", name=f"ep4_{j}") for j in range(8)]
                for j in range(8):
                    psa = ph1pa.tile([128, 2 * D], F32, tag="p_psa")
                    psb = ph1pb.tile([128, D], F32, tag="p_psb")
                    for k in range(8):
                        lhsT = docT[k][:, j * 128:(j + 1) * 128]
                        nc.tensor.matmul(out=psa[:], lhsT=lhsT,
                                         rhs=wcat_sb[k][:, 0:2 * D],
                                         start=(k == 0), stop=(k == 7),
                                         skip_group_check=True)
                        nc.tensor.matmul(out=psb[:], lhsT=lhsT,
                                         rhs=wcat_sb[k][:, 2 * D:3 * D],
                                         start=(k == 0), stop=(k == 7),
                                         skip_group_check=True)
                    p12 = ph1.tile([128, 2 * D], F32, tag="p12")
                    nc.vector.tensor_copy(out=p12[:], in_=psa[:])
                    nc.sync.dma_start(
                        out=t_start[j * 128:(j + 1) * 128, 0:D], in_=p12[:, 0:D]
                    )
                    nc.sync.dma_start(
                        out=t_end[j * 128:(j + 1) * 128, 0:D], in_=p12[:, D:2 * D]
                    )
                    nc.vector.tensor_scalar_mul(ep4[j][:, 0:D], psb[:], e_sb[:, j:j + 1])
                    nc.vector.tensor_copy(out=ep4[j][:, D:D + 1], in_=e_sb[:, j:j + 1])

                # prefix C = LT @ EP4ext (inclusive); exclusive = incl - own
                for m in range(8):
                    psc = ph1c.tile([128, D + 1], F32, tag="c_ps")
                    for k in range(m + 1):
                        nc.tensor.matmul(
                            out=psc[:], lhsT=(lt[:] if k == m else ones_t[:]),
                            rhs=ep4[k][:], start=(k == 0), stop=(k == m),
                        )
                    cinc = ph1.tile([128, D + 1], F32, tag="cinc")
                    nc.vector.tensor_copy(out=cinc[:], in_=psc[:])
                    cexc = ph1.tile([128, D + 1], F32, tag="cexc")
                    nc.vector.tensor_tensor(
                        out=cexc[:], in0=cinc[:], in1=ep4[m][:], op=OP.subtract
                    )
                    nc.sync.dma_start(
                        out=t_end[m * 128:(m + 1) * 128, D:2 * D + 1], in_=cinc[:]
                    )
                    nc.sync.dma_start(
                        out=t_start[m * 128:(m + 1) * 128, D:2 * D + 1], in_=cexc[:]
                    )

                # width table: P3 = wemb @ W3 + b_span
                pst = ph1tr.tile([WD, L], F32, tag="tr")
                nc.tensor.transpose(out=pst[:], in_=wemb_sb[:], identity=ident[:L, :L])
                wembT = ph1.tile([WD, L], F32, tag="wembT")
                nc.vector.tensor_copy(out=wembT[:], in_=pst[:])
                ps3 = ph1c.tile([L, D], F32, tag="c_ps")
                nc.tensor.matmul(out=ps3[:], lhsT=wembT[:], rhs=w3_sb[:],
                                 start=True, stop=True)
                p3 = ph1.tile([L, D], F32, tag="p3")
                nc.vector.tensor_tensor(out=p3[:], in0=ps3[:], in1=bspan_bc[:L, :],
                                        op=OP.add)
                nc.sync.dma_start(out=t_w[:, :], in_=p3[:])

            # ---------------- phase 2: span pipeline ----------------------
            kT = [persist.tile([128, SC], F32, tag=f"kT{c}", name=f"kT{c}") for c in range(2)]
            vsb = [persist.tile([128, D], F32, tag=f"vsb{it}", name=f"vsb{it}")
                   for it in range(NT)]

            with (
                tc.tile_pool(name="ph2", bufs=2) as ph2,
                tc.tile_pool(name="ph2iln", bufs=1) as ph2iln,
                tc.tile_pool(name="ph2tr", bufs=2, space="PSUM") as ph2tr,
                tc.tile_pool(name="ph2v", bufs=1, space="PSUM") as ph2v,
                tc.tile_pool(name="ph2k", bufs=2, space="PSUM") as ph2k,
            ):
                ilnT = [ph2iln.tile([128, SC], F32, tag=f"ilnT{c}", name=f"ilnT{c}") for c in range(2)]
                for it in range(NT):
                    gs = ph2.tile([128, 2 * D + 1], F32, tag="gs")
                    ge = ph2.tile([128, 2 * D + 1], F32, tag="ge")
                    gw = ph2.tile([128, D], F32, tag="gw")
                    nc.gpsimd.indirect_dma_start(
                        out=gs[:], out_offset=None, in_=t_start[:],
                        in_offset=bass.IndirectOffsetOnAxis(
                            ap=idxs_sb[:, it:it + 1], axis=0),
                    )
                    nc.gpsimd.indirect_dma_start(
                        out=ge[:], out_offset=None, in_=t_end[:],
                        in_offset=bass.IndirectOffsetOnAxis(
                            ap=idxe_sb[:, it:it + 1], axis=0),
                    )
                    nc.gpsimd.indirect_dma_start(
                        out=gw[:], out_offset=None, in_=t_w[:],
                        in_offset=bass.IndirectOffsetOnAxis(
                            ap=idxw_sb[:, it:it + 1], axis=0),
                    )
                    den = ph2.tile([128, 1], F32, tag="den")
                    nc.vector.tensor_tensor(out=den[:], in0=ge[:, 2 * D:2 * D + 1],
                                            in1=gs[:, 2 * D:2 * D + 1], op=OP.subtract)
                    rden = ph2.tile([128, 1], F32, tag="rden")
                    nc.vector.reciprocal(out=rden[:], in_=den[:])
                    num = ph2.tile([128, D], F32, tag="num")
                    nc.vector.tensor_tensor(out=num[:], in0=ge[:, D:2 * D],
                                            in1=gs[:, D:2 * D], op=OP.subtract)
                    acc1 = ph2.tile([128, D], F32, tag="acc1")
                    nc.vector.tensor_tensor(out=acc1[:], in0=gs[:, 0:D],
                                            in1=ge[:, 0:D], op=OP.add)
                    acc2 = ph2.tile([128, D], F32, tag="acc2")
                    nc.vector.tensor_tensor(out=acc2[:], in0=acc1[:], in1=gw[:],
                                            op=OP.add)
                    x = ph2.tile([128, D], F32, tag="x")
                    nc.vector.scalar_tensor_tensor(
                        out=x[:], in0=num[:], scalar=rden[:, 0:1], in1=acc2[:],
                        op0=OP.mult, op1=OP.add,
                    )
                    # LayerNorm
                    st6 = ph2.tile([128, 6], F32, tag="st6")
                    nc.vector.bn_stats(out=st6[:], in_=x[:])
                    mv = ph2.tile([128, 2], F32, tag="mv")
                    nc.vector.bn_aggr(out=mv[:], in_=st6[:])
                    sstd = ph2.tile([128, 1], F32, tag="sstd")
                    nc.scalar.activation(out=sstd[:], in_=mv[:, 1:2], func=AF.Sqrt,
                                         bias=eps_sb[:, 0:1])
                    rstd = ph2.tile([128, 1], F32, tag="rstd")
                    nc.vector.reciprocal(out=rstd[:], in_=sstd[:])
                    xn = ph2.tile([128, D], F32, tag="xn")
                    nc.vector.tensor_scalar(
                        out=xn[:], in0=x[:], scalar1=mv[:, 0:1], scalar2=rstd[:, 0:1],
                        op0=OP.subtract, op1=OP.mult,
                    )
                    iln = ph2.tile([128, D], F32, tag="iln")
                    nc.vector.tensor_tensor(out=iln[:], in0=xn[:], in1=g_in_bc[:],
                                            op=OP.mult)
                    nc.vector.tensor_tensor(out=iln[:], in0=iln[:], in1=beta_in_bc[:],
                                            op=OP.add)
                    nc.sync.dma_start(
                        out=inputs_out[it * 128:(it + 1) * 128, :], in_=iln[:]
                    )
                    # transpose into ilnT columns
                    for c in range(2):
                        ps = ph2tr.tile([128, 128], F32, tag="tr")
                        nc.tensor.transpose(out=ps[:],
                                            in_=iln[:, c * 128:(c + 1) * 128],
                                            identity=ident[:])
                        nc.vector.tensor_copy(
                            out=ilnT[c][:, it * 128:(it + 1) * 128], in_=ps[:]
                        )
                    # v = iln @ wv + bv (with ones column appended)
                    psv = ph2v.tile([128, D], F32, tag="v_ps")
                    for c in range(2):
                        nc.tensor.matmul(
                            out=psv[:], lhsT=ilnT[c][:, it * 128:(it + 1) * 128],
                            rhs=wv_sb[c][:], start=(c == 0), stop=(c == 1),
                        )
                    nc.vector.tensor_tensor(out=vsb[it][:], in0=psv[:],
                                            in1=bv_bc[:], op=OP.add)
                    # kT chunk matmuls once a 512-span chunk of ilnT is ready
                    if it in (3, 7, 11, 15, 19, 23, 27, 29):
                        if it == 29:
                            off, cw = 3584, 256
                        else:
                            off, cw = (it - 3) // 4 * 512, 512
                        for m in range(2):
                            psk = ph2k.tile([128, 512], F32, tag="k_ps")
                            for c in range(2):
                                nc.tensor.matmul(
                                    out=psk[:, 0:cw],
                                    lhsT=wk_sb[c][:, m * 128:(m + 1) * 128],
                                    rhs=ilnT[c][:, off:off + cw],
                                    start=(c == 0), stop=(c == 1),
                                )
                            nc.vector.tensor_scalar(
                                out=kT[m][:, off:off + cw], in0=psk[:, 0:cw],
                                scalar1=bkT[m][:, 0:1], scalar2=None, op0=OP.add,
                            )

            # ---------------- phase 3: slot attention ---------------------
            slots = persist.tile([Q, D], F32, tag="slots0")
            nc.sync.dma_start(out=slots[:], in_=slots_q[:, :])

            with (
                tc.tile_pool(name="ph3", bufs=2) as ph3,
                tc.tile_pool(name="ph3w", bufs=4) as ph3w,
                tc.tile_pool(name="ph3d", bufs=2, space="PSUM") as ph3d,
                tc.tile_pool(name="ph3u", bufs=1, space="PSUM") as ph3u,
                tc.tile_pool(name="ph3g", bufs=1, space="PSUM") as ph3g,
                tc.tile_pool(name="ph3tr", bufs=2, space="PSUM") as ph3tr,
                tc.tile_pool(name="dram3", bufs=2, space="DRAM") as dram3,
            ):
                def layer_norm(src, g_bc, b_bc):
                    st = ph3.tile([Q, 6], F32, tag="ln_st")
                    nc.vector.bn_stats(out=st[:], in_=src[:])
                    mv = ph3.tile([Q, 2], F32, tag="ln_mv")
                    nc.vector.bn_aggr(out=mv[:], in_=st[:])
                    sd = ph3.tile([Q, 1], F32, tag="ln_sd")
                    nc.scalar.activation(out=sd[:], in_=mv[:, 1:2], func=AF.Sqrt,
                                         bias=eps_sb[:Q, 0:1])
                    rs = ph3.tile([Q, 1], F32, tag="ln_rs")
                    nc.vector.reciprocal(out=rs[:], in_=sd[:])
                    o = ph3.tile([Q, D], F32, tag="ln_o")
                    nc.vector.tensor_scalar(
                        out=o[:], in0=src[:], scalar1=mv[:, 0:1], scalar2=rs[:, 0:1],
                        op0=OP.subtract, op1=OP.mult,
                    )
                    nc.vector.tensor_tensor(out=o[:], in0=o[:], in1=g_bc[:Q, :],
                                            op=OP.mult)
                    nc.vector.tensor_tensor(out=o[:], in0=o[:], in1=b_bc[:Q, :],
                                            op=OP.add)
                    return o

                def transpose_q(src, ncols, tag):
                    """[Q, ncols] -> ncols/128 tiles of [128, Q]."""
                    outs = []
                    for c in range(ncols // 128):
                        ps = ph3tr.tile([128, Q], F32, tag="tr")
                        nc.tensor.transpose(
                            out=ps[:], in_=src[:, c * 128:(c + 1) * 128],
                            identity=ident[:Q, :Q],
                        )
                        t = ph3w.tile([128, Q], F32, tag=tag)
                        nc.vector.tensor_copy(out=t[:], in_=ps[:])
                        outs.append(t)
                    return outs

                def make_qT(slots_now):
                    ln = layer_norm(slots_now, g_sl_bc, beta_sl_bc)
                    lnT = transpose_q(ln, D, "lnT")
                    qT = []
                    for m in range(2):
                        ps = ph3d.tile([128, Q], F32, tag="d_ps")
                        for c in range(2):
                            nc.tensor.matmul(
                                out=ps[:], lhsT=wq_sb[c][:, m * 128:(m + 1) * 128],
                                rhs=lnT[c][:], start=(c == 0), stop=(c == 1),
                            )
                        t = ph3.tile([128, Q], F32, tag=f"qT{m}")
                        nc.vector.tensor_scalar(
                            out=t[:], in0=ps[:], scalar1=bqT[m][:, 0:1], scalar2=SCALE,
                            op0=OP.add, op1=OP.mult,
                        )
                        qT.append(t)
                    return qT

                for r in range(3):
                    qT = make_qT(slots)
                    psu = ph3u.tile([Q, D + 1], F32, tag="u_ps")
                    for it in range(NT):
                        psd = ph3d.tile([128, Q], F32, tag="d_ps")
                        for c in range(2):
                            nc.tensor.matmul(
                                out=psd[:], lhsT=kT[c][:, it * 128:(it + 1) * 128],
                                rhs=qT[c][:], start=(c == 0), stop=(c == 1),
                            )
                        ex = ph3.tile([128, Q], F32, tag="ex")
                        rsum = ph3.tile([128, 1], F32, tag="rsum")
                        nc.scalar.activation(out=ex[:], in_=psd[:], func=AF.Exp,
                                             accum_out=rsum[:, 0:1])
                        rr = ph3.tile([128, 1], F32, tag="rr")
                        nc.vector.reciprocal(out=rr[:], in_=rsum[:])
                        at = ph3.tile([128, Q], F32, tag="at")
                        nc.vector.tensor_scalar(
                            out=at[:], in0=ex[:], scalar1=rr[:, 0:1],
                            scalar2=ATTN_EPS, op0=OP.mult, op1=OP.add,
                        )
                        nc.tensor.matmul(
                            out=psu[:, 0:D], lhsT=at[:], rhs=vsb[it][:],
                            start=(it == 0), stop=(it == NT - 1),
                            skip_group_check=True,
                        )
                        nc.tensor.matmul(
                            out=psu[:, D:D + 1], lhsT=at[:], rhs=ones_t[:, 0:1],
                            start=(it == 0), stop=(it == NT - 1),
                            skip_group_check=True,
                        )
                    upart = ph3.tile([Q, D + 1], F32, tag="upart")
                    nc.vector.tensor_copy(out=upart[:], in_=psu[:])
                    cc_in = dram3.tile([Q, D + 1], F32, tag="cc_in")
                    cc_out = dram3.tile([Q, D + 1], F32, tag="cc_out")
                    nc.gpsimd.dma_start(out=cc_in[:], in_=upart[:])
                    nc.gpsimd.collective_compute(
                        "AllReduce", OP.add,
                        replica_groups=[list(range(NCORES))],
                        ins=[cc_in.opt()], outs=[cc_out.opt()],
                    )
                    uall = ph3.tile([Q, D + 1], F32, tag="uall")
                    nc.sync.dma_start(out=uall[:], in_=cc_out[:])
                    rden_u = ph3.tile([Q, 1], F32, tag="rden_u")
                    nc.vector.reciprocal(out=rden_u[:], in_=uall[:, D:D + 1])
                    upd = ph3.tile([Q, D], F32, tag="upd")
                    nc.vector.tensor_scalar_mul(upd[:], uall[:, 0:D], rden_u[:, 0:1])

                    # GRU cell
                    updT = transpose_q(upd, D, "updT")
                    hT = transpose_q(slots, D, "hT")
                    t12 = []
                    for wT, bias_bc, srcT, ttag in (
                        (wihT, bih_bc, updT, "t1"), (whhT, bhh_bc, hT, "t2"),
                    ):
                        psg = ph3g.tile([Q, 3 * D], F32, tag="g_ps")
                        for lo, hi in ((0, 2 * D), (2 * D, 3 * D)):
                            for c in range(2):
                                nc.tensor.matmul(
                                    out=psg[:, lo:hi], lhsT=srcT[c][:],
                                    rhs=wT[c][:, lo:hi],
                                    start=(c == 0), stop=(c == 1),
                                )
                        t = ph3.tile([Q, 3 * D], F32, tag=ttag)
                        nc.vector.tensor_tensor(out=t[:], in0=psg[:],
                                                in1=bias_bc[:Q, :], op=OP.add)
                        t12.append(t)
                    t1, t2 = t12
                    rzpre = ph3.tile([Q, 2 * D], F32, tag="rzpre")
                    nc.vector.tensor_tensor(out=rzpre[:], in0=t1[:, 0:2 * D],
                                            in1=t2[:, 0:2 * D], op=OP.add)
                    rz = ph3.tile([Q, 2 * D], F32, tag="rz")
                    nc.scalar.activation(out=rz[:], in_=rzpre[:], func=AF.Sigmoid)
                    npre = ph3.tile([Q, D], F32, tag="npre")
                    nc.vector.tensor_tensor(out=npre[:], in0=rz[:, 0:D],
                                            in1=t2[:, 2 * D:3 * D], op=OP.mult)
                    nc.vector.tensor_tensor(out=npre[:], in0=npre[:],
                                            in1=t1[:, 2 * D:3 * D], op=OP.add)
                    ng = ph3.tile([Q, D], F32, tag="ng")
                    nc.scalar.activation(out=ng[:], in_=npre[:], func=AF.Tanh)
                    dmn = ph3.tile([Q, D], F32, tag="dmn")
                    nc.vector.tensor_tensor(out=dmn[:], in0=slots[:], in1=ng[:],
                                            op=OP.subtract)
                    nc.vector.tensor_tensor(out=dmn[:], in0=rz[:, D:2 * D],
                                            in1=dmn[:], op=OP.mult)
                    snew = ph3.tile([Q, D], F32, tag="snew")
                    nc.vector.tensor_tensor(out=snew[:], in0=ng[:], in1=dmn[:],
                                            op=OP.add)

                    # MLP with residual
                    ffin = layer_norm(snew, g_ff_bc, beta_ff_bc)
                    fT = transpose_q(ffin, D, "fT")
                    ps1 = ph3g.tile([Q, 3 * D], F32, tag="g_ps")
                    for c in range(2):
                        nc.tensor.matmul(out=ps1[:, 0:2 * D], lhsT=fT[c][:],
                                         rhs=wm1_sb[c][:],
                                         start=(c == 0), stop=(c == 1))
                    h1 = ph3.tile([Q, 2 * D], F32, tag="h1")
                    nc.vector.tensor_tensor(out=h1[:], in0=ps1[:, 0:2 * D],
                                            in1=bm1_bc[:Q, :], op=OP.add)
                    nc.vector.tensor_scalar_max(h1[:], h1[:], 0.0)
                    h1T = transpose_q(h1, 2 * D, "h1T")
                    ps2 = ph3g.tile([Q, 3 * D], F32, tag="g_ps")
                    for c in range(4):
                        nc.tensor.matmul(out=ps2[:, 0:D], lhsT=h1T[c][:],
                                         rhs=wm2_sb[c][:],
                                         start=(c == 0), stop=(c == 3))
                    s_out = persist.tile([Q, D], F32, tag=f"slots{r + 1}")
                    nc.vector.tensor_tensor(out=s_out[:], in0=ps2[:, 0:D],
                                            in1=bm2_bc[:Q, :], op=OP.add)
                    nc.vector.tensor_tensor(out=s_out[:], in0=s_out[:], in1=snew[:],
                                            op=OP.add)
                    slots = s_out

                # final logits
                qT = make_qT(slots)
                for it in range(NT):
                    psd = ph3d.tile([128, Q], F32, tag="d_ps")
                    for c in range(2):
                        nc.tensor.matmul(
                            out=psd[:], lhsT=kT[c][:, it * 128:(it + 1) * 128],
                            rhs=qT[c][:], start=(c == 0), stop=(c == 1),
                        )
                    ex = ph3.tile([128, Q], F32, tag="ex")
                    rsum = ph3.tile([128, 1], F32, tag="rsum")
                    nc.scalar.activation(out=ex[:], in_=psd[:], func=AF.Exp,
                                         accum_out=rsum[:, 0:1])
                    rr = ph3.tile([128, 1], F32, tag="rr")
                    nc.vector.reciprocal(out=rr[:], in_=rsum[:])
                    sm = ph3.tile([128, Q], F32, tag="at")
                    nc.vector.tensor_scalar(
                        out=sm[:], in0=ex[:], scalar1=rr[:, 0:1], scalar2=ATTN_EPS,
                        op0=OP.mult, op1=OP.add,
                    )
                    nc.vector.tensor_scalar_min(sm[:], sm[:], 1.0)
                    pst = ph3tr.tile([Q, 128], F32, tag="tr")
                    nc.tensor.transpose(out=pst[:], in_=sm[:], identity=ident[:])
                    lstage = ph3w.tile([Q, 128], F32, tag="lstage")
                    nc.vector.tensor_copy(out=lstage[:], in_=pst[:])
                    nc.sync.dma_start(
                        out=logits_out[:, it * 128:(it + 1) * 128], in_=lstage[:]
                    )

    _split_excess_waits(nc)
    return nc


_NC = None
LAST_RESULTS = None


def _get_nc():
    global _NC
    if _NC is None:
        _NC = build_nc()
    return _NC


def kernel(**inputs):
    f32 = lambda a: np.ascontiguousarray(np.asarray(a, dtype=np.float32))

    doc = f32(inputs["encoded_doc"])[0]                     # [N, H]
    wsp = f32(inputs["w_span_proj"])                        # [3H+WD, D]
    starts = np.asarray(inputs["span_starts"], dtype=np.int64)
    ends = np.asarray(inputs["span_ends"], dtype=np.int64)

    w_cat = np.concatenate(
        [wsp[0:H], wsp[H:2 * H], wsp[2 * H + WD:3 * H + WD]], axis=1
    )                                                       # [H, 3D]
    w3 = np.ascontiguousarray(wsp[2 * H:2 * H + WD])        # [WD, D]

    widx = np.clip(np.minimum(1 + ends - starts, L) - 1, 0, L - 1)
    spad = NCORES * SC
    st_p = np.zeros(spad, np.int32)
    en_p = np.zeros(spad, np.int32)
    wi_p = np.zeros(spad, np.int32)
    st_p[:S] = starts
    en_p[:S] = ends
    wi_p[:S] = widx

    common = dict(
        doc=doc,
        w_cat=f32(w_cat),
        wwa=f32(inputs["w_word_attn"]),
        bwa=f32(inputs["b_word_attn"]),
        wemb=f32(inputs["span_width_embed"]),
        w3=w3,
        b_span=f32(inputs["b_span_proj"]),
        slots_q=f32(inputs["slots_query"]),
        wq=f32(inputs["wq"]), bq=f32(inputs["bq"]),
        wk=f32(inputs["wk"]), bk=f32(inputs["bk"]),
        wv=f32(inputs["wv"]), bv=f32(inputs["bv"]),
        w_ih=f32(inputs["w_ih"]), w_hh=f32(inputs["w_hh"]),
        b_ih=f32(inputs["b_ih"]), b_hh=f32(inputs["b_hh"]),
        w_mlp1=f32(inputs["w_mlp1"]), b_mlp1=f32(inputs["b_mlp1"]),
        w_mlp2=f32(inputs["w_mlp2"]), b_mlp2=f32(inputs["b_mlp2"]),
        g_in=f32(inputs["g_in"]), beta_in=f32(inputs["beta_in"]),
        g_sl=f32(inputs["g_sl"]), beta_sl=f32(inputs["beta_sl"]),
        g_ff=f32(inputs["g_ff"]), beta_ff=f32(inputs["beta_ff"]),
    )

    in_maps = []
    for c in range(NCORES):
        sl = slice(c * SC, (c + 1) * SC)
        m = dict(common)
        # [SC] -> [128, NT] with element [p, t] = idx[t*128 + p]
        m["idx_start"] = np.ascontiguousarray(st_p[sl].reshape(NT, 128).T)
        m["idx_end"] = np.ascontiguousarray(en_p[sl].reshape(NT, 128).T)
        m["idx_w"] = np.ascontiguousarray(wi_p[sl].reshape(NT, 128).T)
        in_maps.append(m)

    nc = _get_nc()
    global LAST_RESULTS
    LAST_RESULTS = run_bass_kernel_spmd(nc, in_maps, list(range(NCORES)))
    res = LAST_RESULTS.results

    coref = np.empty((1, Q, S), np.float32)
    inputs_full = np.empty((1, S, D), np.float32)
    for c in range(NCORES):
        lo = c * SC
        hi = min(lo + SC, S)
        n = hi - lo
        coref[0, :, lo:hi] = res[c]["logits_out"][:, :n]
        inputs_full[0, lo:hi, :] = res[c]["inputs_out"][:n, :]
    return coref, inputs_full
